# revision 1
# baseline (speedup 1.0000x reference)
"""ConvCapsules2d Trainium2 kernel (Bass/Tile), SPMD over 8 NeuronCores.

Full problem:
  poses (16,32,16,14,14) f32, W (32,32,16,3,3) f32
  V[n,b,c,d,f,g,k,l] = W[b,c,d,k,l] * sum_p poses[n,b,p,2f+k,2g+l]
  V: (16,32,32,16,6,6,3,3) f32  (~340 MB full) -> pure store-bandwidth bound.

Sharding: data-parallel over batch N: core i computes n in [2i, 2i+2).

The 2e-2-of-absmax tolerance lets the device emit a low-precision output and
the host restore f32:
  * compute runs in fp16. tensor_tensor fp16 hits the DVE 2x_1p perf mode
    (2 elem/lane/cycle) only when every operand's innermost AP step is +-1 on
    a 2-byte dtype, so the (k,l) axis is padded 9->10: weight rows are
    (d, kl10) and s rows are (fg, kl10), making both broadcast operands
    innermost-dense runs of 10 (20B, 4B-aligned).
  * stores go out via SWDGE (gpsimd) DMA with an fp16->int8 cast: W is
    pre-scaled by 127/absmax(V) on the host (absmax is exact: V factorizes as
    max_{b,kl} max|W| * max|s|), so the int8 quantization error is <=0.5 LSB
    = 0.4% of absmax, well under the 2e-2 gate.  HBM write traffic drops 4x
    vs the f32 baseline (11.8 MB/core incl. kl-pad, stripped on host).
  * DVE is then the bottleneck (8 muls x (58 + 11520/2) cycles ~= 48.5 us
    busy); the unfold is fused into one overlapping-window strided copy.
    Measured 52.0 us steady-state vs the 129.3 us f32 baseline (2.5x).
    A/B-tested and rejected: GPSIMD tensor_mul offload of one group (Q7 rate
    measured ~6.5 cyc/elem, not the modeled 2.6 -> 73 us total) and folding
    the first P-sum level into SWDGE accum_op=add loads (POOL sequencer
    stalls on the serialized accumulate chain -> 57 us).  Both remain behind
    the GP_OFFLOAD / ACCUM_LOAD flags, off.

Per-core layout: SBUF partition q = n*64 + b*2 + mm, output channel
c = 4*g + 2*mm + clo (g in 0..7).  Free dims carry (clo, d, fg, kl10), so the
DRAM side of each store merges (clo, u) into one contiguous 11520-element run
and the AP stays within the 3-dim DMA limit; each of the 8 stores is
(128 part x 11520 elem) with outer spray count 64.
"""
import numpy as np

import concourse.bacc as bacc
import concourse.mybir as mybir
from concourse.tile import TileContext
from concourse.ap import AP
from concourse import bass_utils

# ---- problem constants (hardcoded per contest contract) ----
NTOT, B, P, H = 16, 32, 16, 14
C, D, K, S = 32, 16, 3, 2
F = (H - K) // S + 1          # 6
FF, KK = F * F, K * K         # 36, 9
KL = 10                       # kl padded 9 -> 10 for the DVE 2x mode
NCORES = 8
N = NTOT // NCORES            # 2 batches per core
NPART = 128
M = C // 2                    # 16 c-pairs
MG, MM = 8, 2                 # c = 4*g + 2*mm + clo
UNIT = D * FF * KL            # 5760 padded elements per (partition, c)
HH = H * H
Q4 = P * HH // 4              # 784: poses quarter (4 p-maps)

STORE_INT8 = True             # False -> fp16 stores (no host dequant scale)
GP_OFFLOAD = False             # group 0 multiply on GPSIMD instead of DVE
ACCUM_LOAD = False             # fold first P-sum level into the poses load


def _emit_body(nc, tc, cpool, wpool, opool, poses, Wt, V_ap):
    """One full kernel body; V_ap is the (N, B, C, UNIT) dram target AP."""
    fp16 = mybir.dt.float16

    # ---- poses load folds the first P-reduction level: quarters accumulate
    # into one (128, 784) tile via the SDMA CCE adder.
    acc = wpool.tile([NPART, HH], fp16, tag="acc")
    if ACCUM_LOAD:
        psum4 = wpool.tile([NPART, Q4], fp16, tag="psum4")
        nc.sync.dma_start(out=psum4[:], in_=poses.ap()[:, 0:Q4])
        for j in range(1, 4):
            nc.gpsimd.dma_start(out=psum4[:], in_=poses.ap()[:, j * Q4:(j + 1) * Q4],
                                accum_op=mybir.AluOpType.add)
        W_sb = cpool.tile([NPART, M * D * KL], fp16, tag="wsb")
        nc.sync.dma_start(out=W_sb[:], in_=Wt.ap())
        # ---- finish the P-sum: 784 -> 392 -> 196
        tmp = wpool.tile([NPART, HH * 2], fp16, tag="tmp")
        nc.vector.tensor_add(out=tmp[:], in0=psum4[:, :HH * 2], in1=psum4[:, HH * 2:])
        nc.vector.tensor_add(out=acc[:], in0=tmp[:, :HH], in1=tmp[:, HH:])
    else:
        poses_sb = cpool.tile([NPART, P * HH], fp16, tag="poses")
        nc.sync.dma_start(out=poses_sb[:], in_=poses.ap())
        W_sb = cpool.tile([NPART, M * D * KL], fp16, tag="wsb")
        nc.sync.dma_start(out=W_sb[:], in_=Wt.ap())
        tmp = wpool.tile([NPART, HH * 8], fp16, tag="tmp")
        nc.vector.tensor_add(out=tmp[:, :HH * 8],
                             in0=poses_sb[:, :HH * 8], in1=poses_sb[:, HH * 8:])
        nc.vector.tensor_add(out=tmp[:, :HH * 4],
                             in0=tmp[:, :HH * 4], in1=tmp[:, HH * 4:HH * 8])
        nc.vector.tensor_add(out=tmp[:, :HH * 2],
                             in0=tmp[:, :HH * 2], in1=tmp[:, HH * 2:HH * 4])
        nc.vector.tensor_add(out=acc[:], in0=tmp[:, :HH], in1=tmp[:, HH:HH * 2])

    # ---- unfold to s2 (f, g, kl10) in ONE strided copy: the source AP uses
    # overlapping windows (offset = (2f+k)*14 + (2g+l)); pad lane kl=9 is
    # never read downstream of the host strip, so it can stay garbage.
    s2 = wpool.tile([NPART, FF * KL], fp16, tag="s2")
    a = acc[:]
    src = AP(a.tensor, a.offset,
             [[HH, NPART], [2 * H, F], [2, F], [H, K], [1, K]])
    d_ = s2[:]
    dst = AP(d_.tensor, d_.offset,
             [[FF * KL, NPART], [F * KL, F], [KL, F], [K, K], [1, K]])
    nc.vector.tensor_copy(out=dst, in_=src)

    # ---- multiply + store; group 0 runs on GPSIMD in 8 chunks interleaved
    # with the store issues (all share the POOL instruction stream).
    vap = V_ap.rearrange("n b (g mm clo) u -> g n b mm clo u",
                         g=MG, mm=MM, clo=2)
    w_all = W_sb[:].rearrange("q (g clo d kl) -> q g clo d kl",
                              g=MG, clo=2, d=D)
    s_bc = s2[:].rearrange("q (fg kl) -> q fg kl", kl=KL)[:, None, None, :, :] \
                .broadcast_to((NPART, 2, D, FF, KL))
    if GP_OFFLOAD:
        gout = opool.tile([NPART, 2 * UNIT], fp16, tag="gout")
        gout_v = gout[:].rearrange("q (clo d fg kl) -> q clo d fg kl",
                                   clo=2, d=D, fg=FF)
        s_d4 = s2[:].rearrange("q (fg kl) -> q fg kl", kl=KL)[:, None, :, :] \
                    .broadcast_to((NPART, 4, FF, KL))

    def gchunk(ci):
        if not GP_OFFLOAD:
            return
        # chunk ci: (clo, d-quarter): 4 d's x (fg, kl) with <=4-dim APs for Q7
        clo, dq = divmod(ci, 4)
        dsl = slice(dq * 4, dq * 4 + 4)
        nc.gpsimd.tensor_mul(
            out=gout_v[:, clo, dsl],
            in0=w_all[:, 0, clo, dsl, None, :].broadcast_to((NPART, 4, FF, KL)),
            in1=s_d4)

    def store(g, tile):
        if STORE_INT8:
            nc.gpsimd.dma_start(out=vap[g], in_=tile[:])  # fp16 -> int8 cast
        else:
            nc.sync.dma_start(out=vap[g], in_=tile[:])

    gchunk(0)
    g0 = 1 if GP_OFFLOAD else 0
    for g in range(g0, MG):
        out_t = opool.tile([NPART, 2 * UNIT], fp16, tag="out")
        out_v = out_t[:].rearrange("q (clo d fg kl) -> q clo d fg kl",
                                   clo=2, d=D, fg=FF)
        w_view = w_all[:, g, :, :, None, :].broadcast_to((NPART, 2, D, FF, KL))
        nc.vector.tensor_mul(out=out_v, in0=w_view, in1=s_bc)
        store(g, out_t)
        gchunk(g)  # 8 chunks total: 1 pre-loop + 7 in-loop
    if GP_OFFLOAD:
        store(0, gout)


def _build(nc):
    fp16 = mybir.dt.float16
    out_dt = mybir.dt.int8 if STORE_INT8 else fp16
    poses = nc.dram_tensor("poses", (NPART, P * HH), fp16, kind="ExternalInput")
    Wt = nc.dram_tensor("W", (NPART, M * D * KL), fp16, kind="ExternalInput")
    V = nc.dram_tensor("V", (N, B, C, UNIT), out_dt, kind="ExternalOutput")

    with TileContext(nc) as tc:
        with tc.tile_pool(name="const", bufs=1) as cpool, \
             tc.tile_pool(name="work", bufs=2) as wpool, \
             tc.tile_pool(name="out", bufs=3) as opool:
            _emit_body(nc, tc, cpool, wpool, opool, poses, Wt, V.ap())
    return nc


def _scale(W: np.ndarray, poses: np.ndarray) -> float:
    """Exact absmax of V (in f32 arithmetic): factorizes per (b, k, l)."""
    s = poses.sum(axis=2)                              # (NTOT, B, H, H)
    idx = (np.arange(F) * S)[:, None] + np.arange(K)[None, :]
    su = s[:, :, idx, :]                               # (NTOT,B,F,K,H)
    su = su[:, :, :, :, idx]                           # (NTOT,B,F,K,F,K)
    max_s = np.abs(su).transpose(1, 3, 5, 0, 2, 4).reshape(B, K, K, -1).max(axis=3)
    max_w = np.abs(W).transpose(0, 3, 4, 1, 2).reshape(B, K, K, -1).max(axis=3)
    return float((max_s * max_w).max())


def permute_W(W: np.ndarray) -> np.ndarray:
    """(B, C, D, K, K) f32 (pre-scaled) -> (128, M*D*KL) fp16, kl padded.

    Row q = n*64 + b*2 + mm holds W[b, 4g+2mm+clo, d, k, l] laid out as
    (g, clo, d, kl10).
    """
    Wp = W.reshape(B, MG, MM, 2, D, KK).transpose(0, 2, 1, 3, 4, 5)
    Wpad = np.zeros((B, MM, MG, 2, D, KL), dtype=np.float16)
    Wpad[..., :KK] = Wp.astype(np.float16)
    Wpad = Wpad.reshape(2 * B, M * D * KL)
    return np.ascontiguousarray(np.concatenate([Wpad, Wpad], axis=0))


def dup_poses(poses_shard: np.ndarray) -> np.ndarray:
    """(N, B, P, H, H) core shard -> (128, P*H*H) fp16: row n*64+b*2+mm."""
    flat = poses_shard.astype(np.float16).reshape(N, B, 1, P * HH)
    return np.ascontiguousarray(np.broadcast_to(flat, (N, B, 2, P * HH))
                                .reshape(NPART, P * HH))


_cached_nc = None


def _get_nc():
    global _cached_nc
    if _cached_nc is None:
        nc = bacc.Bacc("TRN2", target_bir_lowering=False)
        _build(nc)
        nc.compile()
        _cached_nc = nc
    return _cached_nc


def run_spmd(poses: np.ndarray, W: np.ndarray, **spmd_kwargs):
    """Shard, run on 8 cores, gather. Returns (V_full f32, BassKernelResults)."""
    poses = np.ascontiguousarray(np.asarray(poses, dtype=np.float32))
    W = np.ascontiguousarray(np.asarray(W, dtype=np.float32))
    assert poses.shape == (NTOT, B, P, H, H), poses.shape
    assert W.shape == (B, C, D, K, K), W.shape
    if STORE_INT8:
        A = _scale(W, poses)
        Wp = permute_W(W * (127.0 / A))
    else:
        A = None
        Wp = permute_W(W)
    nc = _get_nc()
    in_maps = [{"poses": dup_poses(poses[i * N:(i + 1) * N]), "W": Wp}
               for i in range(NCORES)]
    res = bass_utils.run_bass_kernel_spmd(nc, in_maps, core_ids=list(range(NCORES)),
                                          **spmd_kwargs)
    Vq = np.concatenate([r["V"] for r in res.results], axis=0)  # (16,B,C,UNIT)
    Vq = Vq.reshape(NTOT, B, C, D, FF, KL)[..., :KK]
    V = Vq.astype(np.float32)
    if STORE_INT8:
        V *= A / 127.0
    V = np.ascontiguousarray(V.reshape(NTOT, B, C, D, F, F, K, K))
    return V, res


def kernel(poses: np.ndarray, W: np.ndarray) -> np.ndarray:
    import time
    last_err = None
    for attempt in range(3):
        try:
            V, _ = run_spmd(poses, W)
            return V
        except Exception as e:  # transient NRT/axon device errors: poke + retry
            last_err = e
            time.sleep(2.0)
            try:
                import jax, jax.numpy as jnp
                jnp.sum(jnp.ones((8, 8))).block_until_ready()
            except Exception:
                pass
    raise last_err



# revision 2
# speedup vs baseline: 2.3936x; 2.3936x over previous
"""ConvCapsules2d Trainium2 kernel (Bass/Tile), SPMD over 8 NeuronCores.

Full problem:
  poses (16,32,16,14,14) f32, W (32,32,16,3,3) f32
  V[n,b,c,d,f,g,k,l] = W[b,c,d,k,l] * sum_p poses[n,b,p,2f+k,2g+l]
  V: (16,32,32,16,6,6,3,3) f32  (~340 MB full) -> pure store-bandwidth bound.

Sharding: data-parallel over batch N: core i computes n in [2i, 2i+2).

v2 design (HW-probed numbers in parens):
  * SBUF partition q = n*64 + b*2 + fgq, where fgq splits the f axis in two
    (f = 3*fgq + fL).  Row q's output is the 162 "slots" (fL,g,k,l), each a
    512-wide (c,d) vector: V[q, slot*512 + cd].
  * s scalars: one P-reduction (4 fp16 adds) + one strided unfold copy gives
    s2q[q, slot] f32.  The multiply W[b,cd,kl] * s[q,slot] is then a
    tensor_scalar per slot: in0 = W_rep[q, kl*512:+512], scalar = s2q[:,slot].
    On DVE with fp16 out this hits the 4x_2p DVE mode (196 ns/slot measured
    vs 389 ns/slot equivalent for the v1 tensor_tensor path).
  * DMA is the real wall: casting SWDGE stores move ~430-460 GB/s of SOURCE
    bytes (measured), so an fp16-source slot costs ~2x an int8-source slot.
    Balance three producers against the shared DMA bus:
      - DVE fp16-out slots (196 ns/slot) -> gpsimd cast store (2B/elem src)
      - DVE int8-out slots (382 ns/slot) -> sync store (1B/elem src)
      - Act int8-out slots (771 ns/slot, scalar.mul) -> sync store
    Rejected by probe: gpsimd tensor_scalar producer (8.8 us/slot).
  * int8 quantization via host pre-scale of W by 127/absmax(V) (exact absmax:
    V factorizes per (b,k,l)); host dequant after gather.  Error ~0.5%, gate
    is 2e-2 of absmax.
"""
import numpy as np

import concourse.bacc as bacc
import concourse.mybir as mybir
from concourse.tile import TileContext
from concourse.ap import AP
from concourse import bass_utils

# ---- problem constants (hardcoded per contest contract) ----
NTOT, B, P, H = 16, 32, 16, 14
C, D, K, S = 32, 16, 3, 2
F = (H - K) // S + 1          # 6
KK = K * K                    # 9
NCORES = 8
N = NTOT // NCORES            # 2 batches per core
NPART = 128
CD = C * D                    # 512: free size of one slot
FL, G = 3, 6                  # f = 3*fgq + fL, g
NSLOT = FL * G * KK           # 162 slots (fL,g,k,l) per row
ROWB = NSLOT * CD             # 82944 output bytes per row (int8)
HWIN = 8                      # h rows kept per fgq half (2*fL+k in [0,7))
PCOLS = P * HWIN * H          # 1792 poses elems per row
WCOLS = KK * CD               # 4608 W elems per row

STORE_INT8 = True

# ---- engine split knobs (slots): DVE-fp16 | DVE-int8 | Act-int8 ----
P16, Q8, R8 = 55, 62, 45
assert P16 + Q8 + R8 == NSLOT
# store chunk sizes (slots per DMA) per block
CH16, CH8D, CH8A = 14, 16, 15


def _emit_body(nc, tc, cpool, wpool, opool, poses, Wt, V_ap):
    """One full kernel body; V_ap is the (NPART, ROWB) int8 dram target AP."""
    fp16 = mybir.dt.float16
    fp32 = mybir.dt.float32
    int8 = mybir.dt.int8

    poses_sb = cpool.tile([NPART, PCOLS], fp16, tag="poses")
    nc.sync.dma_start(out=poses_sb[:], in_=poses.ap())
    W_sb = cpool.tile([NPART, WCOLS], fp16, tag="wsb")
    nc.sync.dma_start(out=W_sb[:], in_=Wt.ap())

    # ---- P-reduction: 16 p-maps of 112 -> acc[q, 112] (fp16, 2x mode)
    HL = HWIN * H                 # 112
    tmp = wpool.tile([NPART, HL * 8], fp16, tag="tmp")
    nc.vector.tensor_add(out=tmp[:, :HL * 8],
                         in0=poses_sb[:, :HL * 8], in1=poses_sb[:, HL * 8:])
    nc.vector.tensor_add(out=tmp[:, :HL * 4],
                         in0=tmp[:, :HL * 4], in1=tmp[:, HL * 4:HL * 8])
    nc.vector.tensor_add(out=tmp[:, :HL * 2],
                         in0=tmp[:, :HL * 2], in1=tmp[:, HL * 2:HL * 4])
    acc = wpool.tile([NPART, HL], fp16, tag="acc")
    nc.vector.tensor_add(out=acc[:], in0=tmp[:, :HL], in1=tmp[:, HL:HL * 2])

    # ---- unfold to s2q[q, (fL,g,k,l)] f32 in one strided copy
    s2q = wpool.tile([NPART, NSLOT], fp32, tag="s2q")
    a = acc[:]
    src = AP(a.tensor, a.offset,
             [[HL, NPART], [2 * H, FL], [2, G], [H, K], [1, K]])
    d_ = s2q[:]
    dst = AP(d_.tensor, d_.offset,
             [[NSLOT, NPART], [G * KK, FL], [KK, G], [K, K], [1, K]])
    nc.vector.tensor_copy(out=dst, in_=src)

    def wslice(slot):
        kl = slot % KK
        return W_sb[:, kl * CD:(kl + 1) * CD]

    def sscalar(slot):
        return s2q[:, slot:slot + 1]

    # ---- block 1: DVE fp16-out slots [0, P16) -> gpsimd cast stores
    s0 = 0
    while s0 < P16:
        cnt = min(CH16, P16 - s0)
        ot = opool.tile([NPART, CH16 * CD], fp16, tag="o16")
        for j in range(cnt):
            nc.vector.tensor_scalar_mul(
                out=ot[:, j * CD:(j + 1) * CD],
                in0=wslice(s0 + j), scalar1=sscalar(s0 + j))
        nc.gpsimd.dma_start(out=V_ap[:, s0 * CD:(s0 + cnt) * CD],
                            in_=ot[:, :cnt * CD])
        s0 += cnt

    # ---- block 2: DVE int8-out slots [P16, P16+Q8) -> sync stores
    while s0 < P16 + Q8:
        cnt = min(CH8D, P16 + Q8 - s0)
        ot = opool.tile([NPART, CH8D * CD], int8, tag="o8d")
        for j in range(cnt):
            nc.vector.tensor_scalar_mul(
                out=ot[:, j * CD:(j + 1) * CD],
                in0=wslice(s0 + j), scalar1=sscalar(s0 + j))
        nc.sync.dma_start(out=V_ap[:, s0 * CD:(s0 + cnt) * CD],
                          in_=ot[:, :cnt * CD])
        s0 += cnt

    # ---- block 3: Act int8-out slots [P16+Q8, 162) -> sync stores
    while s0 < NSLOT:
        cnt = min(CH8A, NSLOT - s0)
        ot = opool.tile([NPART, CH8A * CD], int8, tag="o8a")
        for j in range(cnt):
            nc.scalar.mul(ot[:, j * CD:(j + 1) * CD],
                          wslice(s0 + j), sscalar(s0 + j))
        nc.sync.dma_start(out=V_ap[:, s0 * CD:(s0 + cnt) * CD],
                          in_=ot[:, :cnt * CD])
        s0 += cnt


def _build(nc):
    fp16 = mybir.dt.float16
    poses = nc.dram_tensor("poses", (NPART, PCOLS), fp16, kind="ExternalInput")
    Wt = nc.dram_tensor("W", (NPART, WCOLS), fp16, kind="ExternalInput")
    V = nc.dram_tensor("V", (NPART, ROWB), mybir.dt.int8, kind="ExternalOutput")

    with TileContext(nc) as tc:
        with tc.tile_pool(name="const", bufs=2) as cpool, \
             tc.tile_pool(name="work", bufs=2) as wpool, \
             tc.tile_pool(name="out", bufs=3) as opool:
            _emit_body(nc, tc, cpool, wpool, opool, poses, Wt, V.ap())
    return nc


def _scale(W: np.ndarray, poses: np.ndarray) -> float:
    """Exact absmax of V (in f32 arithmetic): factorizes per (b, k, l)."""
    s = poses.sum(axis=2)                              # (NTOT, B, H, H)
    idx = (np.arange(F) * S)[:, None] + np.arange(K)[None, :]
    su = s[:, :, idx, :]                               # (NTOT,B,F,K,H)
    su = su[:, :, :, :, idx]                           # (NTOT,B,F,K,F,K)
    max_s = np.abs(su).transpose(1, 3, 5, 0, 2, 4).reshape(B, K, K, -1).max(axis=3)
    max_w = np.abs(W).transpose(0, 3, 4, 1, 2).reshape(B, K, K, -1).max(axis=3)
    return float((max_s * max_w).max())


def permute_W(W: np.ndarray) -> np.ndarray:
    """(B, C, D, K, K) f32 (pre-scaled) -> (128, WCOLS) fp16.

    Row q = n*64 + b*2 + fgq holds W[b, c, d, k, l] laid out as (k, l, c, d).
    """
    Wp = W.transpose(0, 3, 4, 1, 2).reshape(B, WCOLS).astype(np.float16)
    rep = np.broadcast_to(Wp[None, :, None, :], (N, B, 2, WCOLS))
    return np.ascontiguousarray(rep.reshape(NPART, WCOLS))


def dup_poses(poses_shard: np.ndarray) -> np.ndarray:
    """(N, B, P, H, H) core shard -> (128, PCOLS) fp16.

    Row q = n*64 + b*2 + fgq holds poses[n, b, :, 6*fgq : 6*fgq+8, :].
    """
    halves = np.stack([poses_shard[:, :, :, 0:HWIN, :],
                       poses_shard[:, :, :, 6:6 + HWIN, :]], axis=2)
    return np.ascontiguousarray(
        halves.astype(np.float16).reshape(NPART, PCOLS))


_cached_nc = None


def _get_nc():
    global _cached_nc
    if _cached_nc is None:
        nc = bacc.Bacc("TRN2", target_bir_lowering=False)
        _build(nc)
        nc.compile()
        _cached_nc = nc
    return _cached_nc


def run_spmd(poses: np.ndarray, W: np.ndarray, **spmd_kwargs):
    """Shard, run on 8 cores, gather. Returns (V_full f32, BassKernelResults)."""
    poses = np.ascontiguousarray(np.asarray(poses, dtype=np.float32))
    W = np.ascontiguousarray(np.asarray(W, dtype=np.float32))
    assert poses.shape == (NTOT, B, P, H, H), poses.shape
    assert W.shape == (B, C, D, K, K), W.shape
    A = _scale(W, poses)
    Wp = permute_W(W * (127.0 / A))
    nc = _get_nc()
    in_maps = [{"poses": dup_poses(poses[i * N:(i + 1) * N]), "W": Wp}
               for i in range(NCORES)]
    res = bass_utils.run_bass_kernel_spmd(nc, in_maps, core_ids=list(range(NCORES)),
                                          **spmd_kwargs)
    Vq = np.stack([r["V"] for r in res.results], axis=0)   # (8, 128, ROWB)
    Vq = Vq.reshape(NCORES, N, B, 2, FL, G, K, K, C, D)
    Vq = Vq.transpose(0, 1, 2, 8, 9, 3, 4, 5, 6, 7)        # n,b,c,d,fgq,fL,g,k,l
    V = Vq.astype(np.float32) * (A / 127.0)
    V = np.ascontiguousarray(V.reshape(NTOT, B, C, D, F, F, K, K))
    return V, res


def kernel(poses: np.ndarray, W: np.ndarray) -> np.ndarray:
    import time
    last_err = None
    for attempt in range(3):
        try:
            V, _ = run_spmd(poses, W)
            return V
        except Exception as e:  # transient NRT/axon device errors: poke + retry
            last_err = e
            time.sleep(2.0)
            try:
                import jax, jax.numpy as jnp
                jnp.sum(jnp.ones((8, 8))).block_until_ready()
            except Exception:
                pass
    raise last_err
